# revision 1
# baseline (speedup 1.0000x reference)
"""Trainium2 Bass kernel for the GNN decoder (message passing, cond-layernorm).

Sharding: 8 cores = (batch b in {0,1}) x (pnode quarter q in {0..3}).
Each core owns pnode rows [q*16384, (q+1)*16384) of its batch and every edge
whose receiver lands in that range.  Edges are receiver-sorted on the host and
packed into NG groups of G=104 consecutive segments with a fixed budget of
EPG=512 edge slots per group (padded; pad slots have one-hot row == 0 so they
contribute nothing).  Per group the kernel runs the fused edge pipeline
(embed MLP -> cond LN -> gather(sender/receiver feats) -> update MLP ->
cond LN) and accumulates segment sums via one-hot matmuls that produce the
aggregate directly transposed ([F, seg]) so the pnode MLP can consume it
without further transposes.  Cond-norm output affines (1+scale, shift) are
folded into the next matmul's weights on device once per launch.
"""

import math
import os

import numpy as np

import concourse.bass as bass
import concourse.tile as tile
from concourse import bacc
from concourse.tile import add_dep_helper
from concourse import mybir
from concourse.masks import make_identity
from concourse import library_config

F32 = mybir.dt.float32
BF16 = mybir.dt.bfloat16
I16 = mybir.dt.int16

B, NR, NPTOT, E, F, EIN, H, OUT = 2, 16384, 65536, 262144, 128, 4, 16, 4
EPS = 1e-6
NQ = 4                  # pnode quarters per batch
QP = NPTOT // NQ        # pnodes per core (16384)
G = 104                 # segments per group
EPG = 512               # edge slots per group
NG = (QP + G - 1) // G  # groups per core (158)
NEP = NG * EPG          # padded edge slots per core
PB = 512                # pnode block width
NPB = QP // PB          # pnode blocks per core (32)
IPG = EPG // 16         # idx columns per group for ap_gather wrapping (32)

AF = mybir.ActivationFunctionType
ALU = mybir.AluOpType


def _build_nc():
    nc = bacc.Bacc("TRN2", target_bir_lowering=False, debug=False)

    def inp(name, shape, dtype=F32):
        return nc.dram_tensor(name, shape, dtype, kind="ExternalInput")

    efT = inp("efT", [EIN, NEP], BF16)
    sidx = inp("sidx", [128, NG * IPG], I16)
    ridx = inp("ridx", [128, NG * IPG], I16)
    rrel = inp("rrel", [128, NG * 4])
    rnT_d = inp("rnT", [F, NR])
    pnT_d = inp("pnT", [F, QP])
    iota_d = inp("iotaG", [128, G])
    tau_d = inp("tau", [1, 1])
    cnt_d = inp("cnt", [1, NG * G])

    We1 = inp("We1", [EIN, F], BF16)
    be1 = inp("be1", [F, 1])
    We2 = inp("We2", [F, F], BF16)
    be2 = inp("be2", [1, F], BF16)
    Wu1a = inp("Wu1a", [F, F])
    Wu1b = inp("Wu1b", [F, F], BF16)
    Wu1c = inp("Wu1c", [F, F], BF16)
    bu1 = inp("bu1", [F, 1])
    Wu2 = inp("Wu2", [F, F], BF16)
    bu2 = inp("bu2", [1, F], BF16)
    Wp1n = inp("Wp1n", [F, F], BF16)
    Wp1g = inp("Wp1g", [F, F])
    bp1 = inp("bp1", [F, 1])
    Wp2 = inp("Wp2", [F, F], BF16)
    bp2 = inp("bp2", [1, F], BF16)
    Wo1 = inp("Wo1", [F, F])
    bo1 = inp("bo1", [F, 1])
    Wo2 = inp("Wo2", [F, OUT], BF16)
    bo2 = inp("bo2", [1, OUT], BF16)
    # conditioning nets: e(dge embed), u(pdate), p(node).  r is dead code.
    cond_w = {}
    for k in ("e", "u", "p"):
        cond_w[k] = (
            inp(f"C{k}1", [1, H]),
            inp(f"c{k}1", [H, 1]),
            inp(f"C{k}2a", [H, F]),     # scale half of C2
            inp(f"C{k}2b", [H, F]),     # shift half of C2
            inp(f"c{k}2a1", [F, 1]),    # c2[:F] + 1.0
            inp(f"c{k}2b", [F, 1]),     # c2[F:]
        )

    aggS1 = nc.dram_tensor("aggS1", [F, NG * G], BF16)
    aggS2 = nc.dram_tensor("aggS2", [F, NG * G], BF16)
    outT = nc.dram_tensor("outT", [OUT, QP], F32, kind="ExternalOutput")

    from contextlib import ExitStack

    with tile.TileContext(nc) as tc, ExitStack() as ctx:
        singles = ctx.enter_context(tc.tile_pool(name="singles", bufs=1))
        work = ctx.enter_context(tc.tile_pool(name="work", bufs=2))
        small = ctx.enter_context(tc.tile_pool(name="small", bufs=4))
        psB = ctx.enter_context(tc.tile_pool(name="psB", bufs=2, space="PSUM"))
        psA = ctx.enter_context(tc.tile_pool(name="psA", bufs=2, space="PSUM"))
        psT = ctx.enter_context(tc.tile_pool(name="psT", bufs=2, space="PSUM"))
        psS = ctx.enter_context(tc.tile_pool(name="psS", bufs=2, space="PSUM"))

        # ---------- constants & resident tables ----------
        def load(name, dram, shape, dtype=F32):
            t = singles.tile(shape, dtype, tag=name)
            nc.sync.dma_start(out=t[:], in_=dram[:])
            return t

        rnT = load("rnT", rnT_d, [F, NR])
        pnT = load("pnT", pnT_d, [F, QP])
        iota = load("iota", iota_d, [128, G])
        sWe1 = load("We1", We1, [EIN, F], BF16)
        sbe1 = load("be1", be1, [F, 1])
        sWe2 = load("We2", We2, [F, F], BF16)
        sbe2 = load("be2", be2, [1, F], BF16)
        sWu1a = load("Wu1a", Wu1a, [F, F])
        sWu1b = load("Wu1b", Wu1b, [F, F], BF16)
        sWu1c = load("Wu1c", Wu1c, [F, F], BF16)
        sbu1 = load("bu1", bu1, [F, 1])
        sWu2 = load("Wu2", Wu2, [F, F], BF16)
        sbu2 = load("bu2", bu2, [1, F], BF16)
        sWp1n = load("Wp1n", Wp1n, [F, F], BF16)
        sWp1g = load("Wp1g", Wp1g, [F, F])
        sbp1 = load("bp1", bp1, [F, 1])
        sWp2 = load("Wp2", Wp2, [F, F], BF16)
        sbp2 = load("bp2", bp2, [1, F], BF16)
        sWo1 = load("Wo1", Wo1, [F, F])
        sbo1 = load("bo1", bo1, [F, 1])
        sWo2 = load("Wo2", Wo2, [F, OUT], BF16)
        sbo2 = load("bo2", bo2, [1, OUT], BF16)
        stau = load("tau", tau_d, [1, 1])

        relib = nc.gpsimd.load_library(library_config.ap_gather)

        ident = singles.tile([128, 128], F32, tag="ident")
        make_identity(nc, ident[:])
        ident16 = singles.tile([128, 128], BF16, tag="ident16")
        nc.vector.tensor_copy(out=ident16[:], in_=ident[:])
        ones_r = singles.tile([1, PB], BF16, tag="ones_r")
        nc.vector.memset(ones_r[:], 1.0)
        ones_r32 = singles.tile([1, 128], F32, tag="ones_r32")
        nc.vector.memset(ones_r32[:], 1.0)
        ones_c = singles.tile([128, 1], F32, tag="ones_c")
        nc.vector.memset(ones_c[:], 1.0)
        epsb = singles.tile([128, 1], F32, tag="epsb")
        nc.vector.memset(epsb[:], EPS)

        # ---------- conditioning nets (tau -> scale/shift) + weight folds ----------
        cvec = {}
        for k in ("e", "u", "p"):
            C1, c1, C2a, C2b, c2a1, c2b = cond_w[k]
            sC1 = load(f"C{k}1", C1, [1, H])
            sc1 = load(f"c{k}1", c1, [H, 1])
            sC2a = load(f"C{k}2a", C2a, [H, F])
            sC2b = load(f"C{k}2b", C2b, [H, F])
            sc2a1 = load(f"c{k}2a1", c2a1, [F, 1])
            sc2b = load(f"c{k}2b", c2b, [F, 1])

            ph = psS.tile([H, 1], F32, tag="pS")
            nc.tensor.matmul(ph[:], lhsT=sC1[:], rhs=stau[:], start=True, stop=True)
            hs = small.tile([H, 1], F32, tag=f"hs{k}")
            nc.scalar.activation(hs[:], ph[:], AF.Silu, bias=sc1[:], scale=1.0)

            pscale = psS.tile([F, 1], F32, tag="pS")
            nc.tensor.matmul(pscale[:], lhsT=sC2a[:], rhs=hs[:], start=True, stop=True)
            s1p = singles.tile([F, 1], F32, tag=f"s1p{k}")
            nc.vector.tensor_scalar(
                out=s1p[:], in0=pscale[:], scalar1=sc2a1[:], scalar2=None, op0=ALU.add
            )
            pshift = psS.tile([F, 1], F32, tag="pS")
            nc.tensor.matmul(pshift[:], lhsT=sC2b[:], rhs=hs[:], start=True, stop=True)
            shift = singles.tile([F, 1], F32, tag=f"shift{k}")
            nc.vector.tensor_scalar(
                out=shift[:], in0=pshift[:], scalar1=sc2b[:], scalar2=None, op0=ALU.add
            )
            cvec[k] = (s1p, shift)

        s1pe, shifte = cvec["e"]
        s1pu, shiftu = cvec["u"]
        s1pp, shiftp = cvec["p"]

        # fold cond-norm affines into downstream weights
        fWu1a = singles.tile([F, F], BF16, tag="fWu1a")
        nc.vector.tensor_tensor(
            out=fWu1a[:], in0=sWu1a[:], in1=s1pe[:].to_broadcast([F, F]), op=ALU.mult
        )
        pbu1 = psS.tile([F, 1], F32, tag="pS")
        nc.tensor.matmul(pbu1[:], lhsT=sWu1a[:], rhs=shifte[:], start=True, stop=True)
        fbu1 = singles.tile([F, 1], F32, tag="fbu1")
        nc.vector.tensor_scalar(
            out=fbu1[:], in0=pbu1[:], scalar1=sbu1[:], scalar2=None, op0=ALU.add
        )

        fWp1ge = singles.tile([F, F], BF16, tag="fWp1ge")
        nc.vector.tensor_tensor(
            out=fWp1ge[:], in0=sWp1g[:], in1=s1pe[:].to_broadcast([F, F]), op=ALU.mult
        )
        fWp1gu = singles.tile([F, F], BF16, tag="fWp1gu")
        nc.vector.tensor_tensor(
            out=fWp1gu[:], in0=sWp1g[:], in1=s1pu[:].to_broadcast([F, F]), op=ALU.mult
        )
        shifteu = small.tile([F, 1], F32, tag="shifteu")
        nc.vector.tensor_tensor(
            out=shifteu[:], in0=shifte[:], in1=shiftu[:], op=ALU.add
        )
        pbpe = psS.tile([1, F], F32, tag="pS")
        nc.tensor.matmul(pbpe[:], lhsT=shifteu[:], rhs=sWp1g[:], start=True, stop=True)
        bpe_row = singles.tile([1, F], BF16, tag="bpe_row")
        nc.vector.tensor_copy(out=bpe_row[:], in_=pbpe[:])

        fWo1 = singles.tile([F, F], BF16, tag="fWo1")
        nc.vector.tensor_tensor(
            out=fWo1[:], in0=sWo1[:], in1=s1pp[:].to_broadcast([F, F]), op=ALU.mult
        )
        sWo116 = singles.tile([F, F], BF16, tag="sWo116")
        nc.vector.tensor_copy(out=sWo116[:], in_=sWo1[:])
        pbo1 = psS.tile([F, 1], F32, tag="pS")
        nc.tensor.matmul(pbo1[:], lhsT=sWo1[:], rhs=shiftp[:], start=True, stop=True)
        fbo1 = singles.tile([F, 1], F32, tag="fbo1")
        nc.vector.tensor_scalar(
            out=fbo1[:], in0=pbo1[:], scalar1=sbo1[:], scalar2=None, op0=ALU.add
        )

        # layer-norm helper: stats + (x - mu) * rstd applied per 128-col chunk
        def lnorm(psum4, out4, tag):
            """psum4/out4: [128, 4, 128] psum in, sbuf out (normalized)."""
            mv = small.tile([128, 4, 2], F32, tag=f"mv{tag}")
            for c in range(4):
                st6 = small.tile([128, 6], F32, tag=f"st{tag}")
                nc.vector.bn_stats(out=st6[:], in_=psum4[:, c, :])
                nc.vector.bn_aggr(out=mv[:, c, :], in_=st6[:])
            sd = small.tile([128, 4], F32, tag=f"sd{tag}")
            nc.scalar.activation(sd[:], mv[:, :, 1], AF.Sqrt, bias=epsb[:], scale=1.0)
            rstd = small.tile([128, 4], F32, tag=f"rs{tag}")
            nc.vector.reciprocal(rstd[:], sd[:])
            for c in range(4):
                nc.vector.tensor_scalar(
                    out=out4[:, c, :],
                    in0=psum4[:, c, :],
                    scalar1=mv[:, c, 0:1],
                    scalar2=rstd[:, c : c + 1],
                    op0=ALU.subtract,
                    op1=ALU.mult,
                )

        # ---------- edge phase ----------
        flush1, flush2 = [], []
        for g in range(NG):
            efg = work.tile([EIN, EPG], BF16, tag="efg")
            nc.sync.dma_start(out=efg[:], in_=efT[:, g * EPG : (g + 1) * EPG])
            sit = work.tile([128, IPG], I16, tag="sit")
            nc.sync.dma_start(out=sit[:], in_=sidx[:, g * IPG : (g + 1) * IPG])
            rit = work.tile([128, IPG], I16, tag="rit")
            nc.sync.dma_start(out=rit[:], in_=ridx[:, g * IPG : (g + 1) * IPG])
            rrt = work.tile([128, 4], F32, tag="rrt")
            nc.sync.dma_start(out=rrt[:], in_=rrel[:, g * 4 : (g + 1) * 4])

            sf32 = work.tile([128, EPG], F32, tag="sf32")
            gi1 = nc.gpsimd.ap_gather(
                sf32[:], rnT[:], sit[:], channels=128, num_elems=NR, d=1,
                num_idxs=EPG,
            )
            rf32 = work.tile([128, EPG], F32, tag="rf32")
            gi2 = nc.gpsimd.ap_gather(
                rf32[:], pnT[:], rit[:], channels=128, num_elems=QP, d=1,
                num_idxs=EPG,
            )
            for gi in (gi1, gi2):
                add_dep_helper(gi.ins, relib.ins, sync=True,
                               reason="ap_gather needs library 6")
            sf16 = work.tile([128, EPG], BF16, tag="sfT")
            nc.vector.tensor_copy(out=sf16[:], in_=sf32[:])
            rf16 = work.tile([128, EPG], BF16, tag="rfT")
            nc.vector.tensor_copy(out=rf16[:], in_=rf32[:])
            sfT = sf16[:]
            rfT = rf16[:]

            # embed MLP
            pz1 = psB.tile([128, EPG], F32, tag="pB")
            nc.tensor.matmul(pz1[:], lhsT=sWe1[:], rhs=efg[:], start=True, stop=True)
            y1 = work.tile([128, EPG], BF16, tag="y1")
            nc.scalar.activation(y1[:], pz1[:], AF.Silu, bias=sbe1[:], scale=1.0)

            pz2 = psA.tile([128, 4, 128], F32, tag="pA")
            for c in range(4):
                nc.tensor.matmul(
                    pz2[:, c, :],
                    lhsT=y1[:, c * 128 : (c + 1) * 128],
                    rhs=sWe2[:],
                    start=True,
                    stop=False,
                )
                nc.tensor.matmul(
                    pz2[:, c, :], lhsT=ones_r[:, :128], rhs=sbe2[:],
                    start=False, stop=True,
                )
            ln1 = work.tile([128, 4, 128], BF16, tag="ln1")
            lnorm(pz2, ln1, "e")

            # transpose ln1 -> [F, EPG] for the update matmul
            ptr = psT.tile([128, 4, 128], BF16, tag="pT")
            for c in range(4):
                nc.tensor.transpose(ptr[:, c, :], ln1[:, c, :], ident16[:])
            ln1T = work.tile([128, EPG], BF16, tag="ln1T")
            for c in range(4):
                if c % 2 == 0:
                    nc.vector.tensor_copy(
                        out=ln1T[:, c * 128 : (c + 1) * 128], in_=ptr[:, c, :]
                    )
                else:
                    nc.scalar.activation(
                        ln1T[:, c * 128 : (c + 1) * 128], ptr[:, c, :], AF.Copy
                    )

            # update MLP
            pu1 = psB.tile([128, EPG], F32, tag="pB")
            nc.tensor.matmul(pu1[:], lhsT=fWu1a[:], rhs=ln1T[:], start=True, stop=False)
            nc.tensor.matmul(pu1[:], lhsT=sWu1b[:], rhs=sfT[:], start=False, stop=False)
            nc.tensor.matmul(pu1[:], lhsT=sWu1c[:], rhs=rfT[:], start=False, stop=True)
            yu = work.tile([128, EPG], BF16, tag="yu")
            nc.scalar.activation(yu[:], pu1[:], AF.Silu, bias=fbu1[:], scale=1.0)

            pu2 = psA.tile([128, 4, 128], F32, tag="pA")
            for c in range(4):
                nc.tensor.matmul(
                    pu2[:, c, :],
                    lhsT=yu[:, c * 128 : (c + 1) * 128],
                    rhs=sWu2[:],
                    start=True,
                    stop=False,
                )
                nc.tensor.matmul(
                    pu2[:, c, :], lhsT=ones_r[:, :128], rhs=sbu2[:],
                    start=False, stop=True,
                )
            ln2 = work.tile([128, 4, 128], BF16, tag="ln2")
            lnorm(pu2, ln2, "u")

            # one-hot segment aggregation (transposed output)
            oh = work.tile([128, 4, G], BF16, tag="oh")
            for c in range(4):
                nc.vector.tensor_tensor(
                    out=oh[:, c, :],
                    in0=rrt[:, c : c + 1].to_broadcast([128, G]),
                    in1=iota[:],
                    op=ALU.is_equal,
                )
            Sps = psS.tile([128, 2 * G], F32, tag="pS")
            for c in range(4):
                nc.tensor.matmul(
                    Sps[:, 0:G], lhsT=ln1[:, c, :], rhs=oh[:, c, :],
                    start=(c == 0), stop=(c == 3),
                )
            for c in range(4):
                nc.tensor.matmul(
                    Sps[:, G : 2 * G], lhsT=ln2[:, c, :], rhs=oh[:, c, :],
                    start=(c == 0), stop=(c == 3),
                )
            stg = work.tile([128, 2 * G], BF16, tag="stg")
            nc.scalar.activation(stg[:], Sps[:], AF.Copy)
            flush1.append(
                nc.sync.dma_start(out=aggS1[:, g * G : (g + 1) * G], in_=stg[:, 0:G])
            )
            flush2.append(
                nc.sync.dma_start(
                    out=aggS2[:, g * G : (g + 1) * G], in_=stg[:, G : 2 * G]
                )
            )

        # ---------- pnode phase ----------
        for j in range(NPB):
            sl = slice(j * PB, (j + 1) * PB)
            glo = (j * PB) // G
            ghi = min(((j + 1) * PB - 1) // G, NG - 1)
            s1 = work.tile([128, PB], BF16, tag="sfT")
            ld1 = nc.sync.dma_start(out=s1[:], in_=aggS1[:, sl])
            s2 = work.tile([128, PB], BF16, tag="rfT")
            ld2 = nc.sync.dma_start(out=s2[:], in_=aggS2[:, sl])
            for gg in range(glo, ghi + 1):
                add_dep_helper(ld1.ins, flush1[gg].ins, sync=True,
                               reason="agg RAW")
                add_dep_helper(ld2.ins, flush2[gg].ins, sync=True,
                               reason="agg RAW")
            cq = work.tile([1, PB], F32, tag="cq")
            nc.sync.dma_start(out=cq[:], in_=cnt_d[:, sl])

            mx = work.tile([1, PB], F32, tag="mrow")
            nc.vector.tensor_scalar(
                out=mx[:], in0=cq[:], scalar1=1.0, scalar2=None, op0=ALU.max
            )
            inv = work.tile([1, PB], F32, tag="inv")
            nc.vector.reciprocal(inv[:], mx[:])
            m01 = work.tile([1, PB], BF16, tag="mrow")
            nc.vector.tensor_scalar(
                out=m01[:], in0=cq[:], scalar1=1.0, scalar2=None, op0=ALU.min
            )
            pinv = psT.tile([128, PB], F32, tag="pT")
            nc.tensor.matmul(
                pinv[:], lhsT=ones_r32[:], rhs=inv[:], start=True, stop=True
            )
            invb = work.tile([128, PB], F32, tag="y1")
            nc.scalar.activation(invb[:], pinv[:], AF.Copy)

            pn16b = work.tile([128, PB], BF16, tag="pn16b")
            nc.vector.tensor_copy(out=pn16b[:], in_=pnT[:, sl])
            pA = psB.tile([128, PB], F32, tag="pB")
            nc.tensor.matmul(pA[:], lhsT=fWp1ge[:], rhs=s1[:], start=True, stop=False)
            nc.tensor.matmul(pA[:], lhsT=fWp1gu[:], rhs=s2[:], start=False, stop=True)
            tA = work.tile([128, PB], BF16, tag="stg")
            nc.vector.tensor_tensor(out=tA[:], in0=pA[:], in1=invb[:], op=ALU.mult)

            pzp = psB.tile([128, PB], F32, tag="pB")
            nc.tensor.matmul(pzp[:], lhsT=sWp1n[:], rhs=pn16b[:], start=True, stop=False)
            nc.tensor.matmul(pzp[:], lhsT=ident16[:], rhs=tA[:], start=False, stop=False)
            nc.tensor.matmul(pzp[:], lhsT=bpe_row[:], rhs=m01[:], start=False, stop=True)
            yp = work.tile([128, PB], BF16, tag="yu")
            nc.scalar.activation(yp[:], pzp[:], AF.Silu, bias=sbp1[:], scale=1.0)

            pp2 = psA.tile([128, 4, 128], F32, tag="pA")
            for c in range(4):
                nc.tensor.matmul(
                    pp2[:, c, :],
                    lhsT=yp[:, c * 128 : (c + 1) * 128],
                    rhs=sWp2[:],
                    start=True,
                    stop=False,
                )
                nc.tensor.matmul(
                    pp2[:, c, :], lhsT=ones_r[:, :128], rhs=sbp2[:],
                    start=False, stop=True,
                )
            lnp = work.tile([128, 4, 128], BF16, tag="ln1")
            lnorm(pp2, lnp, "p")

            ptr2 = psT.tile([128, 4, 128], BF16, tag="pT")
            for c in range(4):
                nc.tensor.transpose(ptr2[:, c, :], lnp[:, c, :], ident16[:])
            lnpT = work.tile([128, PB], BF16, tag="ln1T")
            for c in range(4):
                if c % 2 == 0:
                    nc.vector.tensor_copy(
                        out=lnpT[:, c * 128 : (c + 1) * 128], in_=ptr2[:, c, :]
                    )
                else:
                    nc.scalar.activation(
                        lnpT[:, c * 128 : (c + 1) * 128], ptr2[:, c, :], AF.Copy
                    )

            pzo = psB.tile([128, PB], F32, tag="pB")
            nc.tensor.matmul(pzo[:], lhsT=fWo1[:], rhs=lnpT[:], start=True, stop=False)
            nc.tensor.matmul(pzo[:], lhsT=sWo116[:], rhs=pn16b[:], start=False, stop=True)
            yo = work.tile([128, PB], BF16, tag="ln2")
            nc.scalar.activation(yo[:], pzo[:], AF.Silu, bias=fbo1[:], scale=1.0)

            po = psS.tile([OUT, PB], F32, tag="pS")
            nc.tensor.matmul(po[:], lhsT=sWo2[:], rhs=yo[:], start=True, stop=False)
            nc.tensor.matmul(po[:], lhsT=sbo2[:], rhs=ones_r[:], start=False, stop=True)
            oc = work.tile([OUT, PB], F32, tag="oh")
            nc.vector.tensor_copy(out=oc[:], in_=po[:])
            nc.sync.dma_start(out=outT[:, sl], in_=oc[:])

    nc.compile()
    return nc


def _wrap_idx(a):
    """[NG*EPG] int16 -> [128, NG*IPG] wrapped for ap_gather."""
    w = a.reshape(NG, IPG, 16).transpose(2, 0, 1).reshape(16, NG * IPG)
    return np.ascontiguousarray(np.tile(w, (8, 1)))


def _prep_core(ef_b, snd_b, rcv_b, rn_b, pn_b, tau_b, q):
    lo = q * QP
    mask = (rcv_b >= lo) & (rcv_b < lo + QP)
    ed = np.nonzero(mask)[0]
    loc = (rcv_b[ed] - lo).astype(np.int64)
    order = np.argsort(loc, kind="stable")
    ed, loc = ed[order], loc[order]
    grp = loc // G
    cnts = np.bincount(grp, minlength=NG)
    assert cnts.max() <= EPG, f"group overflow: {cnts.max()} > {EPG}"
    gstart = np.concatenate([[0], np.cumsum(cnts)[:-1]])
    slot = grp * EPG + (np.arange(len(ed)) - gstart[grp])

    efp = np.zeros((NEP, EIN), np.float32)
    efp[slot] = ef_b[ed]
    sndp = np.zeros(NEP, np.int16)
    sndp[slot] = snd_b[ed].astype(np.int16)
    rcvp = np.zeros(NEP, np.int16)
    rcvp[slot] = loc.astype(np.int16)
    rrel = np.full(NEP, -1.0, np.float32)
    rrel[slot] = (loc - grp * G).astype(np.float32)

    cnt_seg = np.zeros(NG * G, np.float32)
    cnt_seg[: QP] = np.bincount(loc, minlength=QP).astype(np.float32)

    import ml_dtypes

    pn_q = pn_b[lo : lo + QP]
    return {
        "cnt": cnt_seg.reshape(1, NG * G),
        "efT": np.ascontiguousarray(efp.T.astype(ml_dtypes.bfloat16)),
        "sidx": _wrap_idx(sndp),
        "ridx": _wrap_idx(rcvp),
        "rrel": np.ascontiguousarray(
            rrel.reshape(NG, 4, 128).transpose(2, 0, 1).reshape(128, NG * 4)
        ),
        "rnT": np.ascontiguousarray(rn_b.T),
        "pnT": np.ascontiguousarray(pn_q.T),
        "iotaG": np.tile(np.arange(G, dtype=np.float32), (128, 1)),
        "tau": tau_b.reshape(1, 1).astype(np.float32),
    }


def _prep_weights(i):
    w = {
        "We1": i["We1"], "be1": i["be1"].reshape(F, 1), "We2": i["We2"],
        "be2": i["be2"].reshape(1, F),
        "Wu1a": i["Wu1"][0:F], "Wu1b": i["Wu1"][F : 2 * F],
        "Wu1c": i["Wu1"][2 * F : 3 * F],
        "bu1": i["bu1"].reshape(F, 1), "Wu2": i["Wu2"],
        "bu2": i["bu2"].reshape(1, F),
        "Wp1n": i["Wp1"][0:F], "Wp1g": i["Wp1"][F : 2 * F],
        "bp1": i["bp1"].reshape(F, 1), "Wp2": i["Wp2"],
        "bp2": i["bp2"].reshape(1, F),
        "Wo1": i["Wo1"], "bo1": i["bo1"].reshape(F, 1), "Wo2": i["Wo2"],
        "bo2": i["bo2"].reshape(1, OUT),
    }
    for k in ("e", "u", "p"):
        C1, c1 = i[f"C{k}1"], i[f"c{k}1"]
        C2, c2 = i[f"C{k}2"], i[f"c{k}2"]
        w[f"C{k}1"] = C1.reshape(1, H)
        w[f"c{k}1"] = c1.reshape(H, 1)
        w[f"C{k}2a"] = np.ascontiguousarray(C2[:, 0:F])
        w[f"C{k}2b"] = np.ascontiguousarray(C2[:, F : 2 * F])
        w[f"c{k}2a1"] = (c2[0:F] + 1.0).reshape(F, 1)
        w[f"c{k}2b"] = c2[F : 2 * F].reshape(F, 1)
    import ml_dtypes

    bf16_keys = {"We1", "We2", "Wu1b", "Wu1c", "Wu2", "Wp1n", "Wp2", "Wo2",
                 "be2", "bu2", "bp2", "bo2"}
    return {
        k: np.ascontiguousarray(
            v, dtype=ml_dtypes.bfloat16 if k in bf16_keys else np.float32
        )
        for k, v in w.items()
    }


_NC_CACHE = {}


def build_in_maps(inputs):
    i = {k: np.asarray(v) for k, v in inputs.items()}
    w = _prep_weights(i)
    in_maps = []
    for core in range(8):
        b, q = core // NQ, core % NQ
        m = dict(w)
        m.update(
            _prep_core(
                i["edge_features"][b], i["senders"][b], i["receivers"][b],
                i["rnode_features"][b], i["pnode_features"][b], i["tau"][b], q
            )
        )
        in_maps.append(m)
    return in_maps


def get_nc():
    if "nc" not in _NC_CACHE:
        _NC_CACHE["nc"] = _build_nc()
    return _NC_CACHE["nc"]


def assemble(results):
    out = np.zeros((B, NPTOT, OUT), np.float32)
    for core in range(8):
        b, q = core // NQ, core % NQ
        out[b, q * QP : (q + 1) * QP, :] = results[core]["outT"].T
    return out


def kernel(**inputs):
    from concourse.bass_utils import run_bass_kernel_spmd

    nc = get_nc()
    in_maps = build_in_maps(inputs)
    res = run_bass_kernel_spmd(nc, in_maps, list(range(8)))
    return assemble(res.results)


if __name__ == "__main__":
    import reference

    inputs = reference.setup_inputs()
    out = kernel(**{k: np.asarray(v) for k, v in inputs.items()})
    print("out", out.shape, out.dtype)

